# revision 28
# baseline (speedup 1.0000x reference)
"""Trainium2 Bass kernel for the hex-board pattern one-hot encoder.

Reference semantics (see problem): boards (B, 11, 11) in {-1,0,1} ->
out (B, 27, 12, 12) f32 where out[b,p,i,j] = 1 iff the 3-tuple
(P[i,j], P[i,j+1], P[i+1,j]) of the border-padded 13x13 board equals
pattern p (patterns = product([-1,0,1], repeat=3)), with wildcard
corners at (0,0) [elem0], (0,11) [elem1], (11,0) [elem2].

Host prepads each board to the flat 169-elem 13x13 grid (borders are
constants). On device, per position g: idx = 9*P[g] + 3*P[g+1] +
P[g+13] + 13 in 0..26 via contiguous shifted views, then
out[p] = (idx == p) via 27 elementwise compares, plus tiny fix-ups
for the 3 wildcard corner columns.

Pure data parallel across 8 NeuronCores (batch sharding); memory-bound
on the ~510 MB f32 output write.

NB on sync-wait limits: instructions whose operands have >=2 free dims
use the S3D3 encoding which has room for only ONE embedded sync wait
("Too many sync wait commands" in walrus otherwise). All strided ops
here are placed so they need at most one cross-engine wait.
"""

import numpy as np

import concourse.bacc as bacc
import concourse.mybir as mybir
from concourse.mybir import AluOpType
from concourse.tile import TileContext

N_CORES = 8
BATCH = 32768
B_CORE = BATCH // N_CORES  # 4096
T = 4  # boards per partition per macrotile
NPART = 128
NMACRO = B_CORE // (NPART * T)  # 8
PADW = T * 169 + 14  # flat padded boards per partition + shift-read tail

F32 = mybir.dt.float32

# patterns touched by corner fixups (must be on VectorE, same engine as
# the fixup writes): {0,1,2,3,5,6,8} (corner C+A) u {18..20,24..26} (B+A).
# GpSimd is NOT used for compares: its tensor_scalar measures ~9us/op and
# its SBUF-port lock stalls concurrent VectorE ops to the same speed.
# ScalarE computes (idx==p) as Relu(1-(idx-p)^2) in two activations.
ACT_PS = [9, 10, 11, 12, 13, 14, 15]
DVE_PS = [p for p in range(27) if p not in ACT_PS]


def build_nc(nmacro=NMACRO, debug=False):
    nc = bacc.Bacc("TRN2", target_bir_lowering=False, debug=debug)

    # board b_local = ((m*128 + r)*T + t); per-board input row is the
    # 169-elem host-padded 13x13 grid, packed int8 to cut input DMA 4x
    boards_h = nc.dram_tensor(
        "boards", [nmacro, NPART, PADW], mybir.dt.int8, kind="ExternalInput"
    )
    out_h = nc.dram_tensor(
        "out", [nmacro, NPART, T * 27 * 144], F32, kind="ExternalOutput"
    )

    with TileContext(nc) as tc:
        with (
            tc.tile_pool(name="cpool", bufs=1) as cpool,
            tc.tile_pool(name="ppool", bufs=4) as ppool,
            tc.tile_pool(name="gpool", bufs=2) as gpool,
            tc.tile_pool(name="ipool", bufs=2) as ipool,
            tc.tile_pool(name="opool", bufs=3) as opool,
        ):
            # per-partition -p constants for the ScalarE Square bias.
            # GpSimd is otherwise idle and starts instantly, so these 7
            # memsets run there without delaying VectorE's first macrotile.
            negp = cpool.tile([NPART, 27], F32, name="negp")
            for p in ACT_PS:
                nc.gpsimd.memset(negp[:, p : p + 1], float(-p))

            # prefetch int8 input tiles ahead via HWDGE (fast first-byte);
            # ScalarE casts int8->f32 one macrotile before the data is
            # needed (software-pipelined so the cast never gates VectorE).
            p8_tiles, pf_tiles = {}, {}

            def fetch(mi):
                if mi < nmacro and mi not in p8_tiles:
                    P8 = ppool.tile([NPART, PADW], mybir.dt.int8, name="P8")
                    nc.scalar.dma_start(out=P8, in_=boards_h[mi])
                    p8_tiles[mi] = P8

            def cast(mi):
                if mi < nmacro and mi not in pf_tiles:
                    Pf = ppool.tile([NPART, PADW], F32, name="Pf", bufs=3)
                    nc.scalar.copy(Pf, p8_tiles[mi])
                    pf_tiles[mi] = Pf

            for mi in range(4):
                fetch(mi)
            cast(0)
            cast(1)

            for m in range(nmacro):
                Pf = pf_tiles[m]

                # ---- idx over the full flat grid (contiguous ops) ----
                # idxbig[g] = ((3*P[g] + P[g+1])*3 + 13) + P[g+13]
                NG = T * 169
                ib = gpool.tile([NPART, NG], F32, name="ib")
                nc.vector.tensor_scalar(ib, Pf[:, 0:NG], 3.0, None, AluOpType.mult)
                nc.vector.tensor_tensor(ib, ib, Pf[:, 1 : NG + 1], AluOpType.add)
                nc.vector.tensor_scalar(ib, ib, 3.0, 13.0, AluOpType.mult, AluOpType.add)
                nc.vector.tensor_tensor(ib, ib, Pf[:, 13 : NG + 13], AluOpType.add)

                # ---- compact the 12x12 subgrid per board slot ----
                idx = ipool.tile([NPART, T, 144], F32, name="idx")
                ibv = ib.rearrange("p (t a b) -> p t a b", a=13, b=13)
                for t in range(T):
                    nc.vector.tensor_copy(idx[:, t], ibv[:, t, 0:12, 0:12])

                idxf = idx.rearrange("p t f -> p (t f)")

                # ---- 27 one-hot compares, stored in 3 chunks of 9 so the
                # out-DMA starts as soon as the first third is ready ----
                out_t = opool.tile([NPART, T, 27, 144], F32, name="out_t")
                ohv = out_h[m].rearrange("p (t q f) -> p t q f", t=T, q=27, f=144)
                # claim out_t's DMA WAR dep on ScalarE with a 1-free-dim op
                # (multi-wait capable); its own compare overwrites it below.
                c0 = ACT_PS[0]
                nc.scalar.mul(out_t[:, :, c0, 0], out_t[:, :, c0, 0], 0.0)

                # chunk 0: p 0..8 (all DVE) + corner C fixups + corner A p6.
                # For the first macrotile, store in 3 sub-chunks so the
                # first out-DMA starts as early as possible.
                c0_splits = (
                    [(0, 1), (1, 3), (3, 6), (6, 9)] if m == 0 else [(0, 9)]
                )
                for a, b in c0_splits:
                    for p in range(a, b):
                        nc.vector.tensor_scalar(
                            out_t[:, :, p, :], idxf, float(p), None,
                            AluOpType.is_equal,
                        )
                    # corner (11,0) -> pos 132: idx = 4+3d; ones at
                    # p in {3d+3,3d+4,3d+5}; middle (s=1) already right.
                    for mm in range(3):
                        for pb in (3 * mm, 3 * mm + 2):
                            if a <= pb < b:
                                nc.vector.tensor_scalar(
                                    out_t[:, :, pb, 132], idx[:, :, 132],
                                    float(3 * mm + 1), None, AluOpType.is_equal,
                                )
                    if a <= 6 < b:
                        # corner (0,0) -> pos 0: idx=15; ones at {6,15,24}
                        nc.vector.memset(out_t[:, :, 6, 0], 1.0)
                    nc.sync.dma_start(
                        out=ohv[:, :, a:b, :], in_=out_t[:, :, a:b, :]
                    )

                # chunk 1: p 9..15 all on ScalarE; its store is issued from
                # the ScalarE HWDGE ring so no cross-engine wait is needed
                for p in ACT_PS:
                    col = out_t[:, :, p, :]
                    nc.scalar.activation(
                        col, idxf, mybir.ActivationFunctionType.Square,
                        bias=negp[:, p : p + 1], scale=1.0,
                    )
                    nc.scalar.activation(
                        col, col, mybir.ActivationFunctionType.Relu,
                        bias=1.0, scale=-1.0,
                    )
                nc.sync.dma_start(out=ohv[:, :, 9:16, :], in_=out_t[:, :, 9:16, :])

                # chunk 2: p 16..26 (all DVE) + corner B fixups + corner A
                # p24. For the last macrotile, store in sub-chunks so the
                # final drain is short.
                last = m == nmacro - 1
                c2_splits = [(16, 20), (20, 24), (24, 27)] if last else [(16, 27)]
                for a, b in c2_splits:
                    for p in range(a, b):
                        nc.vector.tensor_scalar(
                            out_t[:, :, p, :], idxf, float(p), None,
                            AluOpType.is_equal,
                        )
                    # corner (0,11) -> pos 11: idx = 22+c; ones at
                    # p in {19+c,22+c,25+c}; middle band already right.
                    for k in range(3):
                        for pb in (18 + k, 24 + k):
                            if a <= pb < b:
                                nc.vector.tensor_scalar(
                                    out_t[:, :, pb, 11], idx[:, :, 11],
                                    float(21 + k), None, AluOpType.is_equal,
                                )
                    if a <= 24 < b:
                        nc.vector.memset(out_t[:, :, 24, 0], 1.0)
                    nc.sync.dma_start(
                        out=ohv[:, :, a:b, :], in_=out_t[:, :, a:b, :]
                    )

                # keep the input pipeline primed
                fetch(m + 4)
                cast(m + 2)

    nc.finalize()  # Bacc.compile(): reg alloc + sync-wait splitting
    return nc


def prep_core_input(boards_core):
    """(B_CORE, 11, 11) f32 -> int8 padded flat [NMACRO, NPART, PADW]."""
    n = boards_core.shape[0]
    P = np.zeros((n, 13, 13), dtype=np.int8)
    P[:, 1:12, 1:12] = boards_core.astype(np.int8)
    P[:, 0, 1:12] = 1
    P[:, 12, 1:12] = 1
    P[:, 1:12, 0] = -1
    P[:, 1:12, 12] = -1
    flat = P.reshape(n // T, T * 169)
    out = np.zeros((n // T, PADW), dtype=np.int8)
    out[:, : T * 169] = flat
    return out.reshape(n // (NPART * T), NPART, PADW)


def kernel(boards):
    from concourse.bass_utils import run_bass_kernel_spmd

    boards = np.ascontiguousarray(np.asarray(boards), dtype=np.float32)
    assert boards.shape == (BATCH, 11, 11)

    nc = build_nc()
    in_maps = [
        {"boards": prep_core_input(boards[c * B_CORE : (c + 1) * B_CORE])}
        for c in range(N_CORES)
    ]
    res = run_bass_kernel_spmd(nc, in_maps, core_ids=list(range(N_CORES)))
    out = np.empty((BATCH, 27, 12, 12), dtype=np.float32)
    for c in range(N_CORES):
        out[c * B_CORE : (c + 1) * B_CORE] = res.results[c]["out"].reshape(
            B_CORE, 27, 12, 12
        )
    return out


# revision 29
# speedup vs baseline: 1.0000x; 1.0000x over previous
"""Trainium2 Bass kernel for the hex-board pattern one-hot encoder.

Reference semantics (see problem): boards (B, 11, 11) in {-1,0,1} ->
out (B, 27, 12, 12) f32 where out[b,p,i,j] = 1 iff the 3-tuple
(P[i,j], P[i,j+1], P[i+1,j]) of the border-padded 13x13 board equals
pattern p (patterns = product([-1,0,1], repeat=3)), with wildcard
corners at (0,0) [elem0], (0,11) [elem1], (11,0) [elem2].

Host prepads each board to the flat 169-elem 13x13 grid (borders are
constants). On device, per position g: idx = 9*P[g] + 3*P[g+1] +
P[g+13] + 13 in 0..26 via contiguous shifted views, then
out[p] = (idx == p) via 27 elementwise compares, plus tiny fix-ups
for the 3 wildcard corner columns.

Pure data parallel across 8 NeuronCores (batch sharding); memory-bound
on the ~510 MB f32 output write.

NB on sync-wait limits: instructions whose operands have >=2 free dims
use the S3D3 encoding which has room for only ONE embedded sync wait
("Too many sync wait commands" in walrus otherwise). All strided ops
here are placed so they need at most one cross-engine wait.
"""

import numpy as np

import concourse.bacc as bacc
import concourse.mybir as mybir
from concourse.mybir import AluOpType
from concourse.tile import TileContext

N_CORES = 8
BATCH = 32768
B_CORE = BATCH // N_CORES  # 4096
T = 4  # boards per partition per macrotile
NPART = 128
NMACRO = B_CORE // (NPART * T)  # 8
PADW = T * 169 + 14  # flat padded boards per partition + shift-read tail

F32 = mybir.dt.float32

# patterns touched by corner fixups (must be on VectorE, same engine as
# the fixup writes): {0,1,2,3,5,6,8} (corner C+A) u {18..20,24..26} (B+A).
# GpSimd is NOT used for compares: its tensor_scalar measures ~9us/op and
# its SBUF-port lock stalls concurrent VectorE ops to the same speed.
# ScalarE computes (idx==p) as Relu(1-(idx-p)^2) in two activations.
ACT_PS = [9, 10, 11, 12, 13, 14, 15]
DVE_PS = [p for p in range(27) if p not in ACT_PS]


def build_nc(nmacro=NMACRO, debug=False):
    nc = bacc.Bacc("TRN2", target_bir_lowering=False, debug=debug)

    # board b_local = ((m*128 + r)*T + t); per-board input row is the
    # 169-elem host-padded 13x13 grid, packed int8 to cut input DMA 4x
    boards_h = nc.dram_tensor(
        "boards", [nmacro, NPART, PADW], mybir.dt.int8, kind="ExternalInput"
    )
    out_h = nc.dram_tensor(
        "out", [nmacro, NPART, T * 27 * 144], F32, kind="ExternalOutput"
    )

    with TileContext(nc) as tc:
        with (
            tc.tile_pool(name="cpool", bufs=1) as cpool,
            tc.tile_pool(name="ppool", bufs=4) as ppool,
            tc.tile_pool(name="gpool", bufs=2) as gpool,
            tc.tile_pool(name="ipool", bufs=2) as ipool,
            tc.tile_pool(name="opool", bufs=3) as opool,
        ):
            # per-partition -p constants for the ScalarE Square bias.
            # GpSimd is otherwise idle and starts instantly, so these 7
            # memsets run there without delaying VectorE's first macrotile.
            negp = cpool.tile([NPART, 27], F32, name="negp")
            for p in ACT_PS:
                nc.gpsimd.memset(negp[:, p : p + 1], float(-p))

            # prefetch int8 input tiles ahead via HWDGE (fast first-byte);
            # ScalarE casts int8->f32 one macrotile before the data is
            # needed (software-pipelined so the cast never gates VectorE).
            p8_tiles, pf_tiles = {}, {}

            def fetch(mi):
                if mi < nmacro and mi not in p8_tiles:
                    P8 = ppool.tile([NPART, PADW], mybir.dt.int8, name="P8")
                    nc.scalar.dma_start(out=P8, in_=boards_h[mi])
                    p8_tiles[mi] = P8

            def cast(mi):
                if mi < nmacro and mi not in pf_tiles:
                    Pf = ppool.tile([NPART, PADW], F32, name="Pf", bufs=3)
                    nc.scalar.copy(Pf, p8_tiles[mi])
                    pf_tiles[mi] = Pf

            for mi in range(4):
                fetch(mi)
            cast(0)
            cast(1)

            for m in range(nmacro):
                Pf = pf_tiles[m]

                # ---- idx over the full flat grid (contiguous ops) ----
                # idxbig[g] = ((3*P[g] + P[g+1])*3 + 13) + P[g+13]
                NG = T * 169
                ib = gpool.tile([NPART, NG], F32, name="ib")
                nc.vector.tensor_scalar(ib, Pf[:, 0:NG], 3.0, None, AluOpType.mult)
                nc.vector.tensor_tensor(ib, ib, Pf[:, 1 : NG + 1], AluOpType.add)
                nc.vector.tensor_scalar(ib, ib, 3.0, 13.0, AluOpType.mult, AluOpType.add)
                nc.vector.tensor_tensor(ib, ib, Pf[:, 13 : NG + 13], AluOpType.add)

                # ---- compact the 12x12 subgrid per board slot ----
                idx = ipool.tile([NPART, T, 144], F32, name="idx")
                ibv = ib.rearrange("p (t a b) -> p t a b", a=13, b=13)
                for t in range(T):
                    nc.vector.tensor_copy(idx[:, t], ibv[:, t, 0:12, 0:12])

                idxf = idx.rearrange("p t f -> p (t f)")

                # ---- 27 one-hot compares, stored in 3 chunks of 9 so the
                # out-DMA starts as soon as the first third is ready ----
                out_t = opool.tile([NPART, T, 27, 144], F32, name="out_t")
                ohv = out_h[m].rearrange("p (t q f) -> p t q f", t=T, q=27, f=144)
                # claim out_t's DMA WAR dep on ScalarE with a 1-free-dim op
                # (multi-wait capable); its own compare overwrites it below.
                c0 = ACT_PS[0]
                nc.scalar.mul(out_t[:, :, c0, 0], out_t[:, :, c0, 0], 0.0)

                # chunk 0: p 0..8 (all DVE) + corner C fixups + corner A p6.
                # For the first macrotile, store in 3 sub-chunks so the
                # first out-DMA starts as early as possible.
                c0_splits = (
                    [(0, 1), (1, 3), (3, 6), (6, 9)] if m == 0 else [(0, 9)]
                )
                for a, b in c0_splits:
                    for p in range(a, b):
                        nc.vector.tensor_scalar(
                            out_t[:, :, p, :], idxf, float(p), None,
                            AluOpType.is_equal,
                        )
                    # corner (11,0) -> pos 132: idx = 4+3d; ones at
                    # p in {3d+3,3d+4,3d+5}; middle (s=1) already right.
                    for mm in range(3):
                        for pb in (3 * mm, 3 * mm + 2):
                            if a <= pb < b:
                                nc.vector.tensor_scalar(
                                    out_t[:, :, pb, 132], idx[:, :, 132],
                                    float(3 * mm + 1), None, AluOpType.is_equal,
                                )
                    if a <= 6 < b:
                        # corner (0,0) -> pos 0: idx=15; ones at {6,15,24}
                        nc.vector.memset(out_t[:, :, 6, 0], 1.0)
                    nc.sync.dma_start(
                        out=ohv[:, :, a:b, :], in_=out_t[:, :, a:b, :]
                    )

                # chunk 1: p 9..15 all on ScalarE; its store is issued from
                # the ScalarE HWDGE ring so no cross-engine wait is needed
                for p in ACT_PS:
                    col = out_t[:, :, p, :]
                    nc.scalar.activation(
                        col, idxf, mybir.ActivationFunctionType.Square,
                        bias=negp[:, p : p + 1], scale=1.0,
                    )
                    nc.scalar.activation(
                        col, col, mybir.ActivationFunctionType.Relu,
                        bias=1.0, scale=-1.0,
                    )
                nc.scalar.dma_start(out=ohv[:, :, 9:16, :], in_=out_t[:, :, 9:16, :])

                # chunk 2: p 16..26 (all DVE) + corner B fixups + corner A
                # p24. For the last macrotile, store in sub-chunks so the
                # final drain is short.
                last = m == nmacro - 1
                c2_splits = [(16, 20), (20, 24), (24, 27)] if last else [(16, 27)]
                for a, b in c2_splits:
                    for p in range(a, b):
                        nc.vector.tensor_scalar(
                            out_t[:, :, p, :], idxf, float(p), None,
                            AluOpType.is_equal,
                        )
                    # corner (0,11) -> pos 11: idx = 22+c; ones at
                    # p in {19+c,22+c,25+c}; middle band already right.
                    for k in range(3):
                        for pb in (18 + k, 24 + k):
                            if a <= pb < b:
                                nc.vector.tensor_scalar(
                                    out_t[:, :, pb, 11], idx[:, :, 11],
                                    float(21 + k), None, AluOpType.is_equal,
                                )
                    if a <= 24 < b:
                        nc.vector.memset(out_t[:, :, 24, 0], 1.0)
                    nc.sync.dma_start(
                        out=ohv[:, :, a:b, :], in_=out_t[:, :, a:b, :]
                    )

                # keep the input pipeline primed
                fetch(m + 4)
                cast(m + 2)

    nc.finalize()  # Bacc.compile(): reg alloc + sync-wait splitting
    return nc


def prep_core_input(boards_core):
    """(B_CORE, 11, 11) f32 -> int8 padded flat [NMACRO, NPART, PADW]."""
    n = boards_core.shape[0]
    P = np.zeros((n, 13, 13), dtype=np.int8)
    P[:, 1:12, 1:12] = boards_core.astype(np.int8)
    P[:, 0, 1:12] = 1
    P[:, 12, 1:12] = 1
    P[:, 1:12, 0] = -1
    P[:, 1:12, 12] = -1
    flat = P.reshape(n // T, T * 169)
    out = np.zeros((n // T, PADW), dtype=np.int8)
    out[:, : T * 169] = flat
    return out.reshape(n // (NPART * T), NPART, PADW)


def kernel(boards):
    from concourse.bass_utils import run_bass_kernel_spmd

    boards = np.ascontiguousarray(np.asarray(boards), dtype=np.float32)
    assert boards.shape == (BATCH, 11, 11)

    nc = build_nc()
    in_maps = [
        {"boards": prep_core_input(boards[c * B_CORE : (c + 1) * B_CORE])}
        for c in range(N_CORES)
    ]
    res = run_bass_kernel_spmd(nc, in_maps, core_ids=list(range(N_CORES)))
    out = np.empty((BATCH, 27, 12, 12), dtype=np.float32)
    for c in range(N_CORES):
        out[c * B_CORE : (c + 1) * B_CORE] = res.results[c]["out"].reshape(
            B_CORE, 27, 12, 12
        )
    return out


# revision 30
# speedup vs baseline: 1.1865x; 1.1865x over previous
"""Trainium2 Bass kernel for the hex-board pattern one-hot encoder.

Reference semantics (see problem): boards (B, 11, 11) in {-1,0,1} ->
out (B, 27, 12, 12) f32 where out[b,p,i,j] = 1 iff the 3-tuple
(P[i,j], P[i,j+1], P[i+1,j]) of the border-padded 13x13 board equals
pattern p (patterns = product([-1,0,1], repeat=3)), with wildcard
corners at (0,0) [elem0], (0,11) [elem1], (11,0) [elem2].

Host prepads each board to the flat 169-elem 13x13 grid (borders are
constants). On device, per position g: idx = 9*P[g] + 3*P[g+1] +
P[g+13] + 13 in 0..26 via contiguous shifted views, then
out[p] = (idx == p) via 27 elementwise compares, plus tiny fix-ups
for the 3 wildcard corner columns.

Pure data parallel across 8 NeuronCores (batch sharding); memory-bound
on the ~510 MB f32 output write.

NB on sync-wait limits: instructions whose operands have >=2 free dims
use the S3D3 encoding which has room for only ONE embedded sync wait
("Too many sync wait commands" in walrus otherwise). All strided ops
here are placed so they need at most one cross-engine wait.
"""

import numpy as np

import concourse.bacc as bacc
import concourse.mybir as mybir
from concourse.mybir import AluOpType
from concourse.tile import TileContext

N_CORES = 8
BATCH = 32768
B_CORE = BATCH // N_CORES  # 4096
T = 4  # boards per partition per macrotile
NPART = 128
NMACRO = B_CORE // (NPART * T)  # 8
PADW = T * 169 + 14  # flat padded boards per partition + shift-read tail

F32 = mybir.dt.float32

# patterns touched by corner fixups (must be on VectorE, same engine as
# the fixup writes): {0,1,2,3,5,6,8} (corner C+A) u {18..20,24..26} (B+A).
# GpSimd is NOT used for compares: its tensor_scalar measures ~9us/op and
# its SBUF-port lock stalls concurrent VectorE ops to the same speed.
# ScalarE computes (idx==p) as Relu(1-(idx-p)^2) in two activations.
ACT_PS = [9, 10, 11, 12, 13, 14, 15]
DVE_PS = [p for p in range(27) if p not in ACT_PS]


def build_nc(nmacro=NMACRO, debug=False):
    nc = bacc.Bacc("TRN2", target_bir_lowering=False, debug=debug)

    # board b_local = ((m*128 + r)*T + t); per-board input row is the
    # 169-elem host-padded 13x13 grid, packed int8 to cut input DMA 4x
    boards_h = nc.dram_tensor(
        "boards", [nmacro, NPART, PADW], mybir.dt.int8, kind="ExternalInput"
    )
    out_h = nc.dram_tensor(
        "out", [nmacro, NPART, T * 27 * 144], F32, kind="ExternalOutput"
    )

    with TileContext(nc) as tc:
        with (
            tc.tile_pool(name="cpool", bufs=1) as cpool,
            tc.tile_pool(name="ppool", bufs=4) as ppool,
            tc.tile_pool(name="gpool", bufs=2) as gpool,
            tc.tile_pool(name="ipool", bufs=2) as ipool,
            tc.tile_pool(name="opool", bufs=3) as opool,
        ):
            # per-partition -p constants for the ScalarE Square bias.
            # GpSimd is otherwise idle and starts instantly, so these 7
            # memsets run there without delaying VectorE's first macrotile.
            negp = cpool.tile([NPART, 27], F32, name="negp")
            for p in ACT_PS:
                nc.gpsimd.memset(negp[:, p : p + 1], float(-p))

            # prefetch int8 input tiles ahead via HWDGE (fast first-byte);
            # ScalarE casts int8->f32 one macrotile before the data is
            # needed (software-pipelined so the cast never gates VectorE).
            p8_tiles, pf_tiles = {}, {}

            def fetch(mi):
                if mi < nmacro and mi not in p8_tiles:
                    P8 = ppool.tile([NPART, PADW], mybir.dt.int8, name="P8")
                    nc.scalar.dma_start(out=P8, in_=boards_h[mi])
                    p8_tiles[mi] = P8

            def cast(mi):
                if mi < nmacro and mi not in pf_tiles:
                    Pf = ppool.tile([NPART, PADW], F32, name="Pf", bufs=3)
                    nc.scalar.copy(Pf, p8_tiles[mi])
                    pf_tiles[mi] = Pf

            for mi in range(4):
                fetch(mi)
            cast(0)
            cast(1)

            for m in range(nmacro):
                Pf = pf_tiles[m]

                # ---- idx over the full flat grid (contiguous ops) ----
                # idxbig[g] = ((3*P[g] + P[g+1])*3 + 13) + P[g+13]
                NG = T * 169
                ib = gpool.tile([NPART, NG], F32, name="ib")
                nc.vector.tensor_scalar(ib, Pf[:, 0:NG], 3.0, None, AluOpType.mult)
                nc.vector.tensor_tensor(ib, ib, Pf[:, 1 : NG + 1], AluOpType.add)
                nc.vector.tensor_scalar(ib, ib, 3.0, 13.0, AluOpType.mult, AluOpType.add)
                nc.vector.tensor_tensor(ib, ib, Pf[:, 13 : NG + 13], AluOpType.add)

                # ---- compact the 12x12 subgrid per board slot ----
                idx = ipool.tile([NPART, T, 144], F32, name="idx")
                ibv = ib.rearrange("p (t a b) -> p t a b", a=13, b=13)
                for t in range(T):
                    nc.vector.tensor_copy(idx[:, t], ibv[:, t, 0:12, 0:12])

                idxf = idx.rearrange("p t f -> p (t f)")

                # ---- 27 one-hot compares, stored in 3 chunks of 9 so the
                # out-DMA starts as soon as the first third is ready ----
                out_t = opool.tile([NPART, T, 27, 144], F32, name="out_t")
                ohv = out_h[m].rearrange("p (t q f) -> p t q f", t=T, q=27, f=144)
                # claim out_t's DMA WAR dep on ScalarE with a 1-free-dim op
                # (multi-wait capable); its own compare overwrites it below.
                c0 = ACT_PS[0]
                nc.scalar.mul(out_t[:, :, c0, 0], out_t[:, :, c0, 0], 0.0)

                # chunk 0: p 0..8 (all DVE) + corner C fixups + corner A p6.
                # For the first macrotile, store in 3 sub-chunks so the
                # first out-DMA starts as early as possible.
                c0_splits = (
                    [(0, 1), (1, 3), (3, 6), (6, 9)] if m == 0 else [(0, 9)]
                )
                for a, b in c0_splits:
                    for p in range(a, b):
                        nc.vector.tensor_scalar(
                            out_t[:, :, p, :], idxf, float(p), None,
                            AluOpType.is_equal,
                        )
                    # corner (11,0) -> pos 132: idx = 4+3d; ones at
                    # p in {3d+3,3d+4,3d+5}; middle (s=1) already right.
                    for mm in range(3):
                        for pb in (3 * mm, 3 * mm + 2):
                            if a <= pb < b:
                                nc.vector.tensor_scalar(
                                    out_t[:, :, pb, 132], idx[:, :, 132],
                                    float(3 * mm + 1), None, AluOpType.is_equal,
                                )
                    if a <= 6 < b:
                        # corner (0,0) -> pos 0: idx=15; ones at {6,15,24}
                        nc.vector.memset(out_t[:, :, 6, 0], 1.0)
                    nc.sync.dma_start(
                        out=ohv[:, :, a:b, :], in_=out_t[:, :, a:b, :]
                    )

                # chunk 1: p 9..15 all on ScalarE; its store is issued from
                # the ScalarE HWDGE ring so no cross-engine wait is needed
                for p in ACT_PS:
                    col = out_t[:, :, p, :]
                    nc.scalar.activation(
                        col, idxf, mybir.ActivationFunctionType.Square,
                        bias=negp[:, p : p + 1], scale=1.0,
                    )
                    nc.scalar.activation(
                        col, col, mybir.ActivationFunctionType.Relu,
                        bias=1.0, scale=-1.0,
                    )
                nc.scalar.dma_start(out=ohv[:, :, 9:16, :], in_=out_t[:, :, 9:16, :])

                # chunk 2: p 16..26 (all DVE) + corner B fixups + corner A
                # p24. For the last macrotile, store in sub-chunks so the
                # final drain is short.
                last = m == nmacro - 1
                c2_splits = [(16, 20), (20, 24), (24, 27)] if last else [(16, 27)]
                for a, b in c2_splits:
                    for p in range(a, b):
                        nc.vector.tensor_scalar(
                            out_t[:, :, p, :], idxf, float(p), None,
                            AluOpType.is_equal,
                        )
                    # corner (0,11) -> pos 11: idx = 22+c; ones at
                    # p in {19+c,22+c,25+c}; middle band already right.
                    for k in range(3):
                        for pb in (18 + k, 24 + k):
                            if a <= pb < b:
                                nc.vector.tensor_scalar(
                                    out_t[:, :, pb, 11], idx[:, :, 11],
                                    float(21 + k), None, AluOpType.is_equal,
                                )
                    if a <= 24 < b:
                        nc.vector.memset(out_t[:, :, 24, 0], 1.0)
                    nc.sync.dma_start(
                        out=ohv[:, :, a:b, :], in_=out_t[:, :, a:b, :]
                    )

                # keep the input pipeline primed
                fetch(m + 4)
                cast(m + 2)

    nc.finalize()  # Bacc.compile(): reg alloc + sync-wait splitting
    return nc


def prep_core_input(boards_core):
    """(B_CORE, 11, 11) f32 -> int8 padded flat [NMACRO, NPART, PADW]."""
    n = boards_core.shape[0]
    P = np.zeros((n, 13, 13), dtype=np.int8)
    P[:, 1:12, 1:12] = boards_core.astype(np.int8)
    P[:, 0, 1:12] = 1
    P[:, 12, 1:12] = 1
    P[:, 1:12, 0] = -1
    P[:, 1:12, 12] = -1
    flat = P.reshape(n // T, T * 169)
    out = np.zeros((n // T, PADW), dtype=np.int8)
    out[:, : T * 169] = flat
    return out.reshape(n // (NPART * T), NPART, PADW)


def run_spmd(nc, in_maps):
    """Like bass2jax.run_bass_via_pjrt, but the donated zero output buffers
    are created ON DEVICE (separate jit) instead of being uploaded from the
    host — avoids a ~510MB host->device transfer whose tail can overlap and
    slow down kernel execution."""
    import jax
    import jax.numpy as jnp
    from jax.experimental.shard_map import shard_map
    from jax.sharding import Mesh, NamedSharding, PartitionSpec

    import concourse.mybir as mb
    from concourse import bass2jax

    bass2jax.install_neuronx_cc_hook()
    n_cores = len(in_maps)
    partition_name = nc.partition_id_tensor.name if nc.partition_id_tensor else None

    in_names, out_names, out_avals = [], [], []
    for alloc in nc.m.functions[0].allocations:
        if not isinstance(alloc, mb.MemoryLocationSet):
            continue
        name = alloc.memorylocations[0].name
        if alloc.kind == "ExternalInput":
            if name != partition_name:
                in_names.append(name)
        elif alloc.kind == "ExternalOutput":
            out_names.append(name)
            out_avals.append(
                jax.core.ShapedArray(tuple(alloc.tensor_shape), mb.dt.np(alloc.dtype))
            )
    n_params = len(in_names)
    n_outs = len(out_avals)
    all_names = in_names + out_names
    if partition_name is not None:
        all_names.append(partition_name)

    def _body(*args):
        operands = list(args)
        if partition_name is not None:
            operands.append(bass2jax.partition_id_tensor())
        return tuple(
            bass2jax._bass_exec_p.bind(
                *operands,
                out_avals=tuple(out_avals),
                in_names=tuple(all_names),
                out_names=tuple(out_names),
                lowering_input_output_aliases=(),
                sim_require_finite=True,
                sim_require_nnan=True,
                nc=nc,
            )
        )

    devices = jax.devices()[:n_cores]
    mesh = Mesh(np.asarray(devices), ("core",))
    in_specs = (PartitionSpec("core"),) * (n_params + n_outs)
    out_specs = (PartitionSpec("core"),) * n_outs
    sharded = jax.jit(
        shard_map(
            _body, mesh=mesh, in_specs=in_specs, out_specs=out_specs, check_rep=False
        ),
        donate_argnums=tuple(range(n_params, n_params + n_outs)),
        keep_unused=True,
    )
    concat_in = [
        np.concatenate([np.asarray(in_maps[c][k]) for c in range(n_cores)], axis=0)
        for k in in_names
    ]
    # on-device zero buffers (sharded), no host upload
    zero_fn = jax.jit(
        lambda: tuple(
            jnp.zeros((n_cores * a.shape[0], *a.shape[1:]), a.dtype) for a in out_avals
        ),
        out_shardings=tuple(
            NamedSharding(mesh, PartitionSpec("core")) for _ in out_avals
        ),
    )
    zeros = zero_fn()
    out_arrs = sharded(*concat_in, *zeros)
    return [
        {
            k: np.asarray(out_arrs[i]).reshape(n_cores, *out_avals[i].shape)[c]
            for i, k in enumerate(out_names)
        }
        for c in range(n_cores)
    ]


def kernel(boards):
    boards = np.ascontiguousarray(np.asarray(boards), dtype=np.float32)
    assert boards.shape == (BATCH, 11, 11)

    nc = build_nc()
    in_maps = [
        {"boards": prep_core_input(boards[c * B_CORE : (c + 1) * B_CORE])}
        for c in range(N_CORES)
    ]
    results = run_spmd(nc, in_maps)
    out = np.empty((BATCH, 27, 12, 12), dtype=np.float32)
    for c in range(N_CORES):
        out[c * B_CORE : (c + 1) * B_CORE] = results[c]["out"].reshape(
            B_CORE, 27, 12, 12
        )
    return out
